# revision 36
# baseline (speedup 1.0000x reference)
"""Trainium2 Bass kernel for a hybrid attention+SwiGLU transformer layer.

Strategy: pure data parallelism over B*S = 4096 tokens -> 8 shards of 512.
Each core recomputes K/V over a 1024-token halo (sliding-window attention),
so no collectives are needed. Activations are kept feature-major ("transposed",
[feature, token]) on chip so every projection matmul uses the natural weight
layout as the stationary operand and tokens as the moving free dim (N=512).
Attention scores are computed transposed (scoresT[k, q]) which makes the
whole attention block transpose-free; softmax sums over the partition axis
via ones-matmuls on the PE.

v2: RMSNorm is DEFERRED past the QKV projections (projections are linear in
the per-token scale r): the heavy matmuls consume unnormalized transposed x,
and r is folded into the rope staging muls / V-evacuation, which removes the
serial norm->scale->transpose chain from the PE critical path. Own token
tiles are processed first so Q/KV start early; ctx tiles interleave with the
Q head loop; wo is streamed once in two column passes; PSUM evacuations run
on DVE instead of ACT; the FFN down accumulator is bf16.
"""
import sys, os, math

sys.path.insert(0, '/opt/trn_rl_repo')

import numpy as np
import ml_dtypes

import concourse.bass as bass
import concourse.mybir as mybir
import concourse.tile as tile
from concourse import bacc
from concourse.masks import make_identity
from concourse import bass_isa
from concourse.bass_utils import run_bass_kernel_spmd

AF = mybir.ActivationFunctionType
DT = mybir.dt
ALU = mybir.AluOpType
BF16 = ml_dtypes.bfloat16

N_CORES = 8
EPS = 1e-6
ROPE_BASE = 10000.0
RD = 64           # rotary dim
WINDOW = 1024
EXP_BIAS = -5.0

FULL = dict(D=2048, H=16, KVH=4, FFN=8192, B=2, S=2048, OWN=512, CTX=1536)

# quadrant-local 16-row half swap for stream_shuffle (rope pair exchange)
SHUF_MASK = [(i + 16) % 32 for i in range(32)]
# per-head rotary feature permutation: [e0..e15 | o0..o15 | e16..e31 | o16..o31 | 64:]
ROPE_PERM = ([2 * i for i in range(16)] + [2 * i + 1 for i in range(16)]
             + [32 + 2 * i for i in range(16)] + [33 + 2 * i for i in range(16)]
             + list(range(64, 128)))


def build_program(cfg, timing_iters=None):
    D, H, KVH, FFN = cfg['D'], cfg['H'], cfg['KVH'], cfg['FFN']
    OWN, CTX = cfg['OWN'], cfg['CTX']
    HD = 128
    ND = D // 128            # feature blocks of the model dim
    NF = FFN // 128          # feature blocks of the ffn dim
    NO = OWN // 128          # own token tiles (4)
    NT = CTX // 128          # context token tiles (12)
    NCH = CTX // 512         # context chunks of 512 (3)
    NCTX = NT - NO           # ctx-only tiles (8)
    FG = 16                  # ffn blocks per group
    NFG = NF // FG
    f32, bf16 = DT.float32, DT.bfloat16

    nc = bacc.Bacc("TRN2", target_bir_lowering=False, debug=False,
                   num_devices=N_CORES)
    EXT = "Internal" if timing_iters else "ExternalInput"

    # ---------------- DRAM I/O ----------------
    if timing_iters:
        tick_d = nc.dram_tensor("tick", [1, 4], DT.float32,
                                 kind="ExternalInput")
    x_bf = nc.dram_tensor("x_bf", [CTX, D], bf16, kind=EXT)
    x_f = nc.dram_tensor("x_f", [OWN, D], f32, kind=EXT)
    wq_d = nc.dram_tensor("wq", [H, 128, ND * 128], bf16, kind=EXT)
    wk_d = nc.dram_tensor("wk", [KVH, 128, ND * 128], bf16, kind=EXT)
    wv_d = nc.dram_tensor("wv", [128, ND * KVH * HD], bf16, kind=EXT)
    wo_d = nc.dram_tensor("wo", [H * HD, D], bf16, kind=EXT)
    wg_d = nc.dram_tensor("wg", [NF, 128, ND * 128], bf16, kind=EXT)
    wu_d = nc.dram_tensor("wu", [NF, 128, ND * 128], bf16, kind=EXT)
    wd_d = nc.dram_tensor("wd", [ND, 128, NF * 128], bf16, kind=EXT)
    cosq_d = nc.dram_tensor("cosq", [64, OWN], bf16, kind=EXT)
    sinq_d = nc.dram_tensor("sinq", [64, OWN], bf16, kind=EXT)
    cosk_d = nc.dram_tensor("cosk", [64, CTX], bf16, kind=EXT)
    sink_d = nc.dram_tensor("sink", [64, CTX], bf16, kind=EXT)
    mask_d = nc.dram_tensor("mask", [128, NT * OWN], bf16, kind=EXT)
    y_d = nc.dram_tensor("y", [OWN, D], f32, kind="ExternalOutput")

    rsd = 1.0 / math.sqrt(HD)
    VW = KVH * HD
    KO = (NCH - 1) * 512     # rbk column offset of the own chunk

    from contextlib import ExitStack
    with tile.TileContext(nc) as tc:
        with ExitStack() as ctx:
            pool = lambda *a, **kw: ctx.enter_context(tc.tile_pool(*a, **kw))
            constp = pool(name="const", bufs=1)
            bigA = pool(name="bigA", bufs=1)      # xT_own -> t_fg
            bigB = pool(name="bigB", bufs=1)      # attnT -> gT
            mkacc = pool(name="mkacc", bufs=1)    # masks -> ffn accumulator
            qTp = pool(name="qT", bufs=1)
            kTp = pool(name="kT", bufs=1)
            vPp = pool(name="vP", bufs=1)
            wvresp = pool(name="wvres", bufs=1)
            wpanp = pool(name="wpan", bufs=3)     # streamed weight panels
            hbfp = pool(name="hbf", bufs=2)       # bf16 token tiles
            ropep = pool(name="rope", bufs=2)
            costp = pool(name="cost", bufs=1)     # cos/sin tables
            rrp = pool(name="rr", bufs=1)         # r broadcast tiles
            ppp = pool(name="pp", bufs=3)         # small bf16 [128,OWN] tiles
            osbp = pool(name="osb", bufs=2)
            stgp = pool(name="stg", bufs=2)       # staging
            smlp = pool(name="sml", bufs=2)
            recpp = pool(name="recp", bufs=1)
            psp = pool(name="ps", bufs=8, space="PSUM")
            dramp = pool(name="dram", bufs=1, space="DRAM")

            identity_bf = constp.tile([128, 128], bf16, tag="idb")
            make_identity(nc, identity_bf[:])
            identity_f32 = constp.tile([128, 128], f32, tag="idf")
            make_identity(nc, identity_f32[:])
            ones_col = constp.tile([128, 1], bf16, tag="ones_col")
            nc.gpsimd.memset(ones_col[:], 1.0)
            ones_row = constp.tile([1, 128], bf16, tag="ones_row")
            nc.gpsimd.memset(ones_row[:], 1.0)
            eps_b = constp.tile([128, 1], f32, tag="eps_b")
            nc.gpsimd.memset(eps_b[:], EPS)
            expb = constp.tile([128, 1], f32, tag="expb")
            nc.gpsimd.memset(expb[:], EXP_BIAS)

            x2_dram = dramp.tile([OWN, D], f32, tag="x2")

            # rope tables (DMA deferred until after the first x tiles)
            cosq = costp.tile([64, OWN], bf16, tag="cq")
            sinq = costp.tile([64, OWN], bf16, tag="sq")
            cosk = costp.tile([64, CTX], bf16, tag="ck")
            sink = costp.tile([64, CTX], bf16, tag="sk")

            if timing_iters:
                # Internal "inputs" are uninitialized; fill them with benign
                # constants so timing is not distorted by denormals/NaNs.
                cb = hbfp.tile([128, 2048], bf16, tag="hbf")
                nc.gpsimd.memset(cb[:], 0.01)
                cf = stgp.tile([128, 512], f32, tag="xsm", bufs=4)
                nc.gpsimd.memset(cf[:], 0.01)
                for r in range(NT):
                    nc.sync.dma_start(x_bf[r * 128:(r + 1) * 128, :],
                                      cb[:, :D])
                for r in range(NO):
                    for c in range(D // 512):
                        nc.sync.dma_start(
                            x_f[r * 128:(r + 1) * 128, c * 512:(c + 1) * 512],
                            cf[:, :512])
                for hb in range(H):
                    nc.sync.dma_start(wq_d[hb], cb[:, :ND * 128])
                for kb in range(KVH):
                    nc.sync.dma_start(wk_d[kb], cb[:, :ND * 128])
                def fill_cols(dst, width, rows=128):
                    for c0 in range(0, width, 2048):
                        w = min(2048, width - c0)
                        nc.sync.dma_start(dst[:, c0:c0 + w], cb[:rows, :w])
                fill_cols(wv_d[:], ND * KVH * HD)
                for r in range(D // 128):
                    nc.sync.dma_start(wo_d[r * 128:(r + 1) * 128, :],
                                      cb[:, :D])
                for fb in range(NF):
                    nc.sync.dma_start(wg_d[fb], cb[:, :ND * 128])
                    nc.sync.dma_start(wu_d[fb], cb[:, :ND * 128])
                for ob in range(ND):
                    fill_cols(wd_d[ob], NF * 128)
                nc.sync.dma_start(cosq_d[:], cb[:64, :OWN])
                nc.sync.dma_start(sinq_d[:], cb[:64, :OWN])
                nc.sync.dma_start(cosk_d[:], cb[:64, :CTX])
                nc.sync.dma_start(sink_d[:], cb[:64, :CTX])
                fill_cols(mask_d[:], NT * OWN)

            # resident weights: wv (DMA deferred; tile persistent)
            wv_sb = wvresp.tile([128, ND * VW], bf16, tag="wv")

            from contextlib import nullcontext
            loop_ctx = (tc.For_i(0, timing_iters, 1)
                        if timing_iters else nullcontext())
            with loop_ctx:
                hT = bigA.tile([128, ND * CTX], bf16, tag="bigA")
                qT = qTp.tile([128, H * OWN], bf16, tag="qT")
                kT = kTp.tile([128, KVH * CTX], bf16, tag="kT")
                vP = vPp.tile([128, NT * KVH * HD], bf16, tag="vP")
                kpan_all = bigB.tile([128, KVH * ND * 128], bf16, tag="bigB",
                                     name="kpan_all")
                masks = mkacc.tile([128, NT * OWN], bf16, tag="mkacc")
                # r plumbing: r_row holds 1/rms per token (row layout);
                # rbk broadcasts it down all 128 partitions, f32.
                r_row = rrp.tile([1, CTX], bf16, tag="r_row")
                rbk = rrp.tile([128, CTX], bf16, tag="rbk")
                rr_t = rrp.tile([128, NT], f32, tag="rr_t")  # per-tile r cols

                # ---- per-tile: load x, compute r, cast+transpose x ----
                def p1_tile(i):
                    # transposed load of x straight into hT (XBAR DMA transpose)
                    dst = hT.rearrange("p (db c) -> p db c", db=ND)[
                        :, :, i * 128:(i + 1) * 128]
                    nc.sync.dma_start_transpose(dst, x_bf[i * 128:(i + 1) * 128, :])
                    # token-major rows for the r (sum-of-squares) path
                    xb = hbfp.tile([128, D], bf16, tag="hbf")
                    nc.sync.dma_start(xb[:], x_bf[i * 128:(i + 1) * 128, :])
                    # r path (square in 512-col chunks; small scratch)
                    ssp = smlp.tile([128, 4], f32, tag="ssp")
                    for c in range(4):
                        sq = hbfp.tile([128, 512], bf16, tag="sq", bufs=1)
                        nc.scalar.activation(sq[:], xb[:, c * 512:(c + 1) * 512],
                                             AF.Square,
                                             accum_out=ssp[:, c:c + 1])
                    ss = smlp.tile([128, 1], f32, tag="ss")
                    nc.vector.tensor_reduce(ss[:], ssp[:],
                                            mybir.AxisListType.XYZW, ALU.add)
                    sr = smlp.tile([128, 1], f32, tag="sr")
                    nc.scalar.activation(sr[:], ss[:], AF.Sqrt, scale=1.0 / D,
                                         bias=eps_b[:])
                    nc.vector.reciprocal(rr_t[:, i:i + 1], sr[:])
                    pr = psp.tile([1, 128], f32, tag="ps")
                    nc.tensor.transpose(pr[:], rr_t[:, i:i + 1], identity_f32[:])
                    nc.vector.tensor_copy(r_row[0:1, i * 128:(i + 1) * 128],
                                          pr[:])

                def bcast_r(ch):
                    pb = psp.tile([128, 512], f32, tag="ps")
                    nc.tensor.matmul(pb[:], ones_row[:],
                                     r_row[0:1, ch * 512:(ch + 1) * 512],
                                     start=True, stop=True)
                    nc.vector.tensor_copy(rbk[:, ch * 512:(ch + 1) * 512], pb[:])

                # ============ own tiles first ============
                for i in range(NCTX, NT):
                    p1_tile(i)
                bcast_r(NCH - 1)     # own tokens = last chunk of rbk
                # rope q tables early (needed by head 0); the rest is deferred
                nc.sync.dma_start(cosq[:], cosq_d[:])
                nc.sync.dma_start(sinq[:], sinq_d[:])

                # ========= Q heads interleaved with ctx tiles =========
                def q_head(hb):
                    pan = wpanp.tile([128, ND * 128], bf16, tag="wpan")
                    nc.sync.dma_start(pan[:], wq_d[hb])
                    pq = psp.tile([128, OWN], f32, tag="ps")
                    for db in range(ND):
                        nc.tensor.matmul(
                            pq[:], pan[:, db * 128:(db + 1) * 128],
                            hT[:, db * CTX + CTX - OWN: db * CTX + CTX],
                            start=(db == 0), stop=(db == ND - 1))
                    qsl = qT[:, hb * OWN:(hb + 1) * OWN]
                    qstage = ropep.tile([64, OWN], bf16, tag="rst")
                    nc.vector.tensor_mul(qstage[:], pq[0:64, :],
                                         rbk[0:64, KO:KO + OWN])
                    shuf = ropep.tile([64, OWN], bf16, tag="rsh")
                    nc.vector.stream_shuffle(shuf[:], qstage[:], SHUF_MASK)
                    t1 = ropep.tile([64, OWN], bf16, tag="rt1", bufs=1)
                    nc.vector.tensor_mul(t1[:], qstage[:], cosq[:])
                    t2 = ropep.tile([64, OWN], bf16, tag="rt2", bufs=1)
                    nc.vector.tensor_mul(t2[:], shuf[:], sinq[:])
                    nc.vector.tensor_add(qsl[0:64, :], t1[:], t2[:])
                    nc.vector.tensor_mul(qsl[64:128, :], pq[64:128, :],
                                         rbk[64:128, KO:KO + OWN])

                for hb in range(H):
                    q_head(hb)
                    if hb == 3:
                        # deferred prologue DMAs (not needed before kv/attn)
                        nc.sync.dma_start(cosk[:], cosk_d[:])
                        nc.sync.dma_start(sink[:], sink_d[:])
                        nc.sync.dma_start(wv_sb[:], wv_d[:])
                        for kb in range(KVH):
                            nc.sync.dma_start(
                                kpan_all[:, kb * ND * 128:(kb + 1) * ND * 128],
                                wk_d[kb])
                    if hb >= 5 and hb % 2 == 1:
                        p1_tile((hb - 5) // 2)
                        if hb == 11:
                            bcast_r(0)
                for i in range(6, NCTX):
                    p1_tile(i)
                bcast_r(1)

                nc.sync.dma_start(masks[:], mask_d[:])

                # ---- attention helpers (used interleaved with kv below) ----
                attnT = bigB.tile([128, H * OWN], bf16, tag="bigB")
                REP = H // KVH
                attn_state = {}

                def attn_tiles(hb, ts, ti0, mid_cb=None):
                    kb = hb // REP
                    if hb not in attn_state:
                        ap = psp.tile([128, OWN], f32, tag="ps",
                                      name=f"ap{hb}")
                        ssum = psp.tile([128, OWN], f32, tag="ps",
                                        name=f"ssum{hb}")
                        attn_state[hb] = (ap, ssum)
                    ap, ssum = attn_state[hb]
                    ng = NT // 4
                    if ti0 == 0:
                        nc.vector.memset(ssum[:], 0.0)
                    for g in range(len(ts) // 4):
                        gi = ti0 // 4 + g
                        if g == 1 and mid_cb is not None:
                            mid_cb()
                        ts4 = ts[g * 4:(g + 1) * 4]
                        pms = []
                        for t in ts4:
                            sp = psp.tile([128, OWN], f32, tag="ps")
                            nc.tensor.matmul(sp[:],
                                             kT[:, kb * CTX + t * 128:
                                                kb * CTX + (t + 1) * 128],
                                             qT[:, hb * OWN:(hb + 1) * OWN],
                                             start=True, stop=True)
                            pt = ppp.tile([128, OWN], bf16, tag="pt")
                            nc.scalar.activation(pt[:], sp[:], AF.Exp,
                                                 scale=rsd, bias=expb[:])
                            pm = ppp.tile([128, OWN], bf16, tag="pm",
                                          bufs=5)
                            nc.vector.tensor_mul(pm[:], pt[:],
                                                 masks[:, t * OWN:(t + 1) * OWN])
                            pms.append(pm)
                        for gt, t in enumerate(ts4):
                            nc.tensor.matmul(ap[:],
                                             vP[:, t * VW + kb * HD:
                                                t * VW + (kb + 1) * HD],
                                             pms[gt][:],
                                             start=(gi == 0 and gt == 0),
                                             stop=(gi == ng - 1 and gt == 3))
                        # softmax denominators: 4 concurrent M=1 col-tiles
                        for c in range(4):
                            nc.tensor.matmul(ssum[32 * c:32 * c + 1, :],
                                             ones_col[:], pms[c][:],
                                             start=(gi == 0), stop=(gi == ng - 1),
                                             tile_position=(0, 32 * c))

                def attn_norm(hb):
                    ap, ssum = attn_state[hb]
                    s4 = recpp.tile([128, OWN], bf16, tag="s4", bufs=1)
                    nc.vector.tensor_copy(s4[:], ssum[:])
                    st = psp.tile([1, OWN], f32, tag="ps", name=f"st{hb}")
                    nc.tensor.matmul(st[:], ones_col[:], s4[:],
                                     start=True, stop=True)
                    rec = recpp.tile([1, OWN], f32, tag="rec", bufs=1)
                    nc.vector.reciprocal(rec[:], st[:])
                    recb = recpp.tile([1, OWN], bf16, tag="recb", bufs=1)
                    nc.vector.tensor_copy(recb[:], rec[:])
                    pb = psp.tile([128, OWN], f32, tag="ps")
                    nc.tensor.matmul(pb[:], ones_row[:], recb[:],
                                     start=True, stop=True)
                    asb = osbp.tile([128, OWN], bf16, tag="osb")
                    nc.vector.tensor_copy(asb[:], ap[:])
                    nc.vector.tensor_mul(attnT[:, hb * OWN:(hb + 1) * OWN],
                                         asb[:], pb[:])

                # ============ K + V (own chunk first, then 0/1 streamed) =======
                for ch in [NCH - 1] + list(range(NCH - 1)):
                    pk = [psp.tile([128, 512], f32, tag="ps", name=f"pk{ch}_{kb}")
                          for kb in range(KVH)]
                    pv = [psp.tile([128, VW], f32, tag="ps", name=f"pv{ch}_{mi}")
                          for mi in range(4)]
                    for db in range(ND):
                        hb0 = db * CTX + ch * 512
                        for kb in range(KVH):
                            nc.tensor.matmul(
                                pk[kb][:],
                                kpan_all[:, (kb * ND + db) * 128:
                                         (kb * ND + db + 1) * 128],
                                hT[:, hb0:hb0 + 512],
                                start=(db == 0), stop=(db == ND - 1))
                    # K rope drains on DVE while the V matmuls run below
                    for kb in range(KVH):
                        c0 = ch * 512
                        ksl = kT[:, kb * CTX + c0: kb * CTX + c0 + 512]
                        kstage = ropep.tile([64, 512], bf16, tag="rst")
                        nc.vector.tensor_mul(kstage[:], pk[kb][0:64, :],
                                             rbk[0:64, c0:c0 + 512])
                        shuf = ropep.tile([64, 512], bf16, tag="rsh")
                        nc.vector.stream_shuffle(shuf[:], kstage[:], SHUF_MASK)
                        t1 = ropep.tile([64, 512], bf16, tag="rt1", bufs=1)
                        nc.vector.tensor_mul(t1[:], kstage[:],
                                             cosk[:, c0:c0 + 512])
                        t2 = ropep.tile([64, 512], bf16, tag="rt2", bufs=1)
                        nc.vector.tensor_mul(t2[:], shuf[:],
                                             sink[:, c0:c0 + 512])
                        nc.vector.tensor_add(ksl[0:64, :], t1[:], t2[:])
                        nc.vector.tensor_mul(ksl[64:128, :], pk[kb][64:128, :],
                                             rbk[64:128, c0:c0 + 512])
                    for db in range(ND):
                        hb0 = db * CTX + ch * 512
                        for mi in range(4):
                            nc.tensor.matmul(pv[mi][:],
                                             hT[:, hb0 + mi * 128:
                                                hb0 + (mi + 1) * 128],
                                             wv_sb[:, db * VW:(db + 1) * VW],
                                             start=(db == 0), stop=(db == ND - 1))
                    for mi in range(4):
                        t_idx = ch * 4 + mi
                        nc.vector.tensor_scalar_mul(
                            vP[:, t_idx * VW:(t_idx + 1) * VW], pv[mi][:],
                            rr_t[:, t_idx:t_idx + 1])
                    if ch == NCH - 2:
                        # last-emitted chunk: fill the PE while the V/rope
                        # evacuations drain on DVE — head 0's own-chunk tiles
                        # only need kT/vP of the own chunk (long done).
                        attn_tiles(0, list(range(NCTX, NT)), 0)

                # ============ attention (norm pipelined by one head) ===========
                # (attnT reuses the kpan_all slot; K panels are dead now)
                t_order = list(range(NCTX, NT)) + list(range(NCTX))
                for hb in range(H):
                    cb = (lambda h=hb: attn_norm(h - 1)) if hb > 0 else None
                    if hb == 0:
                        attn_tiles(0, t_order[NO:], NO)  # own tiles done in kv
                    else:
                        attn_tiles(hb, t_order, 0, mid_cb=cb)
                attn_norm(H - 1)

                # ====== O projection + residual + transposed x2 build ======
                # wo streamed once: two passes over 1024-wide column halves;
                # PSUM holds 4 mt x 2 dc = 8 banks per pass. Evacuation also
                # builds x2T (transposed unnormalized x2, bf16; FFN rmsnorm is
                # deferred into the SwiGLU elementwise stage) and the per-mt
                # sum-of-squares partials for r2.
                x2T = qTp.tile([128, ND * OWN], bf16, tag="qT")
                sspart = rrp.tile([128, 4 * NO], f32, tag="sspart")
                for dcp in range(2):
                    pos = [[psp.tile([128, 512], f32, tag="ps",
                                     name=f"po{dcp}_{mt}_{dc2}")
                            for dc2 in range(2)] for mt in range(NO)]
                    xs_t = {}
                    for mt in range(NO):
                        for dc2 in range(2):
                            c0 = dcp * 1024 + dc2 * 512
                            xs = stgp.tile([128, 512], f32, tag="xsm",
                                           bufs=4, name=f"xs{dcp}_{mt}_{dc2}")
                            nc.sync.dma_start(
                                xs[:], x_f[mt * 128:(mt + 1) * 128,
                                           c0:c0 + 512])
                            xs_t[mt, dc2] = xs
                    for hb in range(H):
                        pan = wpanp.tile([128, 1024], bf16, tag="wpan")
                        nc.sync.dma_start(
                            pan[:], wo_d[hb * 128:(hb + 1) * 128,
                                         dcp * 1024:(dcp + 1) * 1024])
                        for mt in range(NO):
                            a_sl = attnT[:, hb * OWN + mt * 128:
                                         hb * OWN + (mt + 1) * 128]
                            for dc2 in range(2):
                                nc.tensor.matmul(
                                    pos[mt][dc2][:], a_sl,
                                    pan[:, dc2 * 512:(dc2 + 1) * 512],
                                    start=(hb == 0), stop=(hb == H - 1))
                    for mt in range(NO):
                        for dc2 in range(2):
                            c0 = dcp * 1024 + dc2 * 512
                            xs = xs_t[mt, dc2]
                            x2s = stgp.tile([128, 512], f32, tag="x2s")
                            nc.vector.tensor_add(x2s[:], pos[mt][dc2][:], xs[:])
                            nc.sync.dma_start(
                                x2_dram[mt * 128:(mt + 1) * 128,
                                        c0:c0 + 512], x2s[:])
                            sqp = hbfp.tile([128, 512], bf16, tag="sqp",
                                            bufs=1)
                            nc.scalar.activation(
                                sqp[:], x2s[:], AF.Square,
                                accum_out=sspart[:, mt * 4 + dcp * 2 + dc2:
                                                 mt * 4 + dcp * 2 + dc2 + 1])
                            x2b = ppp.tile([128, 512], bf16, tag="pt")
                            nc.scalar.copy(x2b[:], x2s[:])
                            ptg = psp.tile([128, 512], bf16, tag="ps")
                            for k in range(4):
                                nc.tensor.transpose(
                                    ptg[:, k * 128:(k + 1) * 128],
                                    x2b[:, k * 128:(k + 1) * 128],
                                    identity_bf[:])
                                db = c0 // 128 + k
                                nc.vector.tensor_copy(
                                    x2T[:, db * OWN + mt * 128:
                                        db * OWN + (mt + 1) * 128],
                                    ptg[:, k * 128:(k + 1) * 128])

                # r2 chain: reuse r_row/rbk space (dead after kv/q phases)
                rr2c = rrp.tile([128, NO], f32, tag="rr2c")
                for mt in range(NO):
                    ss2 = smlp.tile([128, 1], f32, tag="ss")
                    nc.vector.tensor_reduce(ss2[:],
                                            sspart[:, mt * 4:(mt + 1) * 4],
                                            mybir.AxisListType.XYZW, ALU.add)
                    sr2 = smlp.tile([128, 1], f32, tag="sr")
                    nc.scalar.activation(sr2[:], ss2[:], AF.Sqrt,
                                         scale=1.0 / D, bias=eps_b[:])
                    nc.vector.reciprocal(rr2c[:, mt:mt + 1], sr2[:])
                    pr2 = psp.tile([1, 128], f32, tag="ps")
                    nc.tensor.transpose(pr2[:], rr2c[:, mt:mt + 1],
                                        identity_f32[:])
                    nc.vector.tensor_copy(r_row[0:1, mt * 128:(mt + 1) * 128],
                                          pr2[:])
                pb2 = psp.tile([128, 512], f32, tag="ps")
                nc.tensor.matmul(pb2[:], ones_row[:], r_row[0:1, 0:OWN],
                                 start=True, stop=True)
                nc.vector.tensor_copy(rbk[:, 0:OWN], pb2[:])

                # ============ FFN gate/up/down (norm folded in) ============
                acc = mkacc.tile([128, ND * OWN], bf16, tag="mkacc")
                for fg in range(NFG):
                    t_fg = bigA.tile([128, FG * OWN], bf16, tag="bigA")
                    for j in range(FG):
                        fb = fg * FG + j
                        gpan = wpanp.tile([128, ND * 128], bf16, tag="wpan")
                        nc.sync.dma_start(gpan[:], wg_d[fb])
                        upan = wpanp.tile([128, ND * 128], bf16, tag="wpan")
                        nc.sync.dma_start(upan[:], wu_d[fb])
                        pg = psp.tile([128, OWN], f32, tag="ps")
                        pu = psp.tile([128, OWN], f32, tag="ps")
                        for db in range(ND):
                            nc.tensor.matmul(pg[:], gpan[:, db * 128:(db + 1) * 128],
                                             x2T[:, db * OWN:(db + 1) * OWN],
                                             start=(db == 0), stop=(db == ND - 1))
                            nc.tensor.matmul(pu[:], upan[:, db * 128:(db + 1) * 128],
                                             x2T[:, db * OWN:(db + 1) * OWN],
                                             start=(db == 0), stop=(db == ND - 1))
                        ab = ppp.tile([128, OWN], bf16, tag="pt")
                        nc.vector.tensor_mul(ab[:], pg[:], rbk[:, 0:OWN])
                        sg = osbp.tile([128, OWN], bf16, tag="osb")
                        nc.scalar.activation(sg[:], ab[:], AF.Sigmoid)
                        m1 = ppp.tile([128, OWN], bf16, tag="pm",
                                      bufs=5)
                        nc.vector.tensor_mul(m1[:], sg[:], ab[:])
                        m2 = ppp.tile([128, OWN], bf16, tag="pt")
                        nc.vector.tensor_mul(m2[:], pu[:], rbk[:, 0:OWN])
                        nc.vector.tensor_mul(t_fg[:, j * OWN:(j + 1) * OWN],
                                             m1[:], m2[:])
                    for ob in range(ND):
                        dpan = wpanp.tile([128, FG * 128], bf16, tag="wpan")
                        nc.sync.dma_start(
                            dpan[:], wd_d[ob, :, fg * FG * 128:(fg + 1) * FG * 128])
                        pd = psp.tile([128, OWN], f32, tag="ps")
                        for j in range(FG):
                            nc.tensor.matmul(pd[:], dpan[:, j * 128:(j + 1) * 128],
                                             t_fg[:, j * OWN:(j + 1) * OWN],
                                             start=(j == 0), stop=(j == FG - 1))
                        osl = acc[:, ob * OWN:(ob + 1) * OWN]
                        if fg == 0:
                            nc.vector.tensor_copy(osl, pd[:])
                        else:
                            nc.vector.tensor_add(osl, osl, pd[:])
                        # final residual interleaved into the last group
                        if fg == NFG - 1 and ob % 4 == 3:
                            og = ob // 4
                            for mt in range(NO):
                                ptf = psp.tile([128, 512], bf16, tag="ps")
                                for k in range(4):
                                    ob2 = og * 4 + k
                                    nc.tensor.transpose(
                                        ptf[:, k * 128:(k + 1) * 128],
                                        acc[:, ob2 * OWN + mt * 128:
                                            ob2 * OWN + (mt + 1) * 128],
                                        identity_bf[:])
                                xs = stgp.tile([128, 512], f32, tag="xsm",
                                               bufs=4)
                                nc.sync.dma_start(
                                    xs[:], x2_dram[mt * 128:(mt + 1) * 128,
                                                   og * 512:(og + 1) * 512])
                                ys = stgp.tile([128, 512], f32, tag="x2s")
                                nc.vector.tensor_add(ys[:], ptf[:], xs[:])
                                nc.sync.dma_start(
                                    y_d[mt * 128:(mt + 1) * 128,
                                        og * 512:(og + 1) * 512], ys[:])

    nc.compile()
    return nc


# ---------------------------------------------------------------------------
# Host-side preparation
# ---------------------------------------------------------------------------

def _rope_tables(pos, dtype=BF16):
    """Build the [64, m] A (cos) and B (+-sin) tables for the permuted layout."""
    inv_freq = 1.0 / (ROPE_BASE ** (np.arange(0, RD, 2, dtype=np.float64) / RD))
    ang = inv_freq[:, None] * pos[None, :].astype(np.float64)   # [32, m]
    cos, sin = np.cos(ang), np.sin(ang)
    rmap = np.concatenate([np.arange(16), np.arange(16),
                           np.arange(16, 32), np.arange(16, 32)])
    sign = np.ones(64); sign[0:16] = -1.0; sign[32:48] = -1.0
    A = cos[rmap]                       # [64, m]
    B = sign[:, None] * sin[rmap]
    return A.astype(dtype), B.astype(dtype)


def prep_inputs(cfg, x, position_ids, attn_norm_w, wq, wk, wv, wo, ffn_norm_w,
                w_gate, w_up, w_down):
    D, H, KVH, FFN = cfg['D'], cfg['H'], cfg['KVH'], cfg['FFN']
    B, S, OWN, CTX = cfg['B'], cfg['S'], cfg['OWN'], cfg['CTX']
    HD = 128
    ND, NF, NT = D // 128, FFN // 128, CTX // 128
    NCHUNK = S // OWN

    x = np.asarray(x, np.float32)
    anw = np.asarray(attn_norm_w, np.float32)
    fnw = np.asarray(ffn_norm_w, np.float32)
    perm = np.asarray(ROPE_PERM)

    def panelize(w, nout):
        # w: [D_in, NOUT*128] -> [NOUT, 128, ND_in*128] panel image
        # pan[ob, p, n*128+c] = w[n*128+p, ob*128+c]
        din = w.shape[0]
        ndin = din // 128
        return np.ascontiguousarray(
            w.reshape(ndin, 128, nout, 128).transpose(2, 1, 0, 3)
            .reshape(nout, 128, ndin * 128))

    wq_f = (np.asarray(wq, np.float32) * anw[:, None]).reshape(D, H, HD)
    wq_f = wq_f[:, :, perm].reshape(D, H * HD)
    wq_t = panelize(wq_f, H).astype(BF16)
    wk_f = (np.asarray(wk, np.float32) * anw[:, None]).reshape(D, KVH, HD)
    wk_f = wk_f[:, :, perm].reshape(D, KVH * HD)
    wk_t = panelize(wk_f, KVH).astype(BF16)
    VW = KVH * HD
    wv_f = np.asarray(wv, np.float32) * anw[:, None]
    wv_t = np.ascontiguousarray(
        wv_f.reshape(ND, 128, VW).transpose(1, 0, 2)
        .reshape(128, ND * VW)).astype(BF16)
    wo_t = np.ascontiguousarray(np.asarray(wo, np.float32)).astype(BF16)
    wg_t = panelize(np.asarray(w_gate, np.float32) * fnw[:, None], NF).astype(BF16)
    wu_t = panelize(np.asarray(w_up, np.float32) * fnw[:, None], NF).astype(BF16)
    wd_t = panelize(np.asarray(w_down, np.float32), ND).astype(BF16)

    pos_ids = np.asarray(position_ids)

    in_maps = []
    for s in range(N_CORES):
        b, c = divmod(s, NCHUNK)
        lo = c * OWN - (CTX - OWN)          # global start of ctx window
        x_c = np.zeros((CTX, D), np.float32)
        g0, g1 = max(0, lo), c * OWN + OWN
        x_c[g0 - lo: g1 - lo] = x[b, g0:g1]

        posq = np.asarray(pos_ids[b, c * OWN: c * OWN + OWN], np.float64)
        posk_idx = np.clip(np.arange(lo, lo + CTX), 0, S - 1)
        posk = np.asarray(pos_ids[b], np.float64)[posk_idx]
        cosq, sinq = _rope_tables(posq)
        cosk, sink = _rope_tables(posk)

        j = np.arange(CTX)[:, None]         # local key index
        qi = np.arange(OWN)[None, :]
        valid = (j >= qi + 1) & (j <= qi + WINDOW) & (j >= (g0 - lo))
        mask = np.ascontiguousarray(
            valid.astype(BF16).reshape(NT, 128, OWN).transpose(1, 0, 2)
            .reshape(128, NT * OWN))

        in_maps.append(dict(
            x_bf=x_c.astype(BF16), x_f=np.ascontiguousarray(x_c[CTX - OWN:]),
            wq=wq_t, wk=wk_t, wv=wv_t, wo=wo_t,
            wg=wg_t, wu=wu_t, wd=wd_t,
            cosq=cosq, sinq=sinq, cosk=cosk, sink=sink, mask=mask))
    return in_maps


_NC_CACHE = {}


def _get_nc(cfg_key='full'):
    if cfg_key not in _NC_CACHE:
        _NC_CACHE[cfg_key] = build_program(FULL)
    return _NC_CACHE[cfg_key]


def kernel(**inputs):
    cfg = FULL
    nc = _get_nc('full')
    in_maps = prep_inputs(cfg, **inputs)
    res = run_bass_kernel_spmd(nc, in_maps, list(range(N_CORES)))
    B, S, D, OWN = cfg['B'], cfg['S'], cfg['D'], cfg['OWN']
    NCHUNK = S // OWN
    out = np.empty((B, S, D), np.float32)
    for s in range(N_CORES):
        b, c = divmod(s, NCHUNK)
        out[b, c * OWN:(c + 1) * OWN] = res.results[s]["y"]
    return out
